# revision 15
# baseline (speedup 1.0000x reference)
"""Sparse (sliding-window) attention Trainium2 kernel, v2.

Problem (hardcoded): B=32, N=1024 tokens on a 16x64 grid, C=256, 8 heads,
head_dim=32, 7x11 sliding window. y = softmax(q k^T/sqrt(d) + mask) v @ Wp + b.
Data-parallel over batch: 4 items per core on 8 cores.

v2 changes vs v1 (267us baseline):
  - Softmax denominator folded into the PV matmul: V tiles carry a ones
    column per head (lhsT [128,33], two heads per PSUM od tile at
    tile_position (0,0)/(0,64)), so the separate ones-lhsT denominator
    matmuls (half the PV stream) disappear.
  - PV accumulates over q-halves (512 wide) instead of quarters: one
    matmul per (chunk, head) half-overlap, start=True on the first chunk
    (whole-bank clear replaces DVE memsets of od).
  - Normalization: one DVE reciprocal per od tile, K=1 selector matmuls
    broadcast the den-reciprocal rows (32/96) across each head's 32
    output partitions (M=33 so the den row itself becomes exactly 1.0 in
    aT), one DVE multiply od*bcast -> aT. The 1.0 rows let the proj bias
    fold into wp (row 32 of the (g=0,s=0) tile), removing bias matmuls.
  - aT keeps junk partitions (33-63, 97-127); host-side wp rows for those
    are zero. One-time PSUM memsets at init keep the junk finite.
  - PSUM->SBUF copies moved off DVE: qkv/v on ScalarE, y on GpSimd.
"""

import contextlib

import numpy as np
import ml_dtypes

import concourse.bass as bass
import concourse.bacc as bacc
import concourse.mybir as mybir
import concourse.tile as tile
from concourse import bass_utils

F32 = mybir.dt.float32
BF16 = mybir.dt.bfloat16
AF = mybir.ActivationFunctionType

H_MAP, W_MAP = 16, 64
N_TOK = H_MAP * W_MAP            # 1024
DIM = 256
HEADS = 8
HDIM = 32
B_FULL = 32
N_CORES = 8
B_LOC = B_FULL // N_CORES        # 4
NCHUNK = N_TOK // 128            # 8 k-chunks (2 grid rows each)
HALF = 512
VE_W = 33 * HEADS                # 264: per-head 32 V dims + ones col


def _qband(c):
    """Valid q range (start token, width) for k-chunk c (rows 2c, 2c+1)."""
    qlo = max(0, 2 * c - 3)
    qhi = min(H_MAP - 1, 2 * c + 4)
    return qlo * W_MAP, (qhi - qlo + 1) * W_MAP


def _half_chunks(h):
    out = []
    for c in range(NCHUNK):
        qs, wc = _qband(c)
        if qs < 512 * (h + 1) and qs + wc > 512 * h:
            out.append(c)
    return out


def build_program(loop_n=1):
    nc = bacc.Bacc("TRN2", target_bir_lowering=False, debug=False)

    xt_d = nc.dram_tensor("xt", [B_LOC, DIM, N_TOK], BF16, kind="ExternalInput")
    wqkvT_d = nc.dram_tensor("wqkvT", [DIM, 2 * DIM], BF16, kind="ExternalInput")
    wvT_d = nc.dram_tensor("wvT", [DIM, VE_W], BF16, kind="ExternalInput")
    wpT_d = nc.dram_tensor("wpT", [4, 128, DIM], BF16, kind="ExternalInput")
    maskc_d = nc.dram_tensor("maskc", [NCHUNK, 128, 512], BF16, kind="ExternalInput")
    y_d = nc.dram_tensor("y", [B_LOC, N_TOK, DIM], F32, kind="ExternalOutput")

    xt = xt_d.ap()
    y = y_d.ap()

    with tile.TileContext(nc) as tc:
        with (
            tc.tile_pool(name="const", bufs=1) as const,
            tc.tile_pool(name="xtp", bufs=4) as xtp,
            tc.tile_pool(name="qkvp", bufs=8) as qkvp,
            tc.tile_pool(name="vp", bufs=16) as vp,
            tc.tile_pool(name="ptp", bufs=8) as ptp,
            tc.tile_pool(name="atp", bufs=4) as atp,
            tc.tile_pool(name="rcp", bufs=2) as rcp,
            tc.tile_pool(name="bcp", bufs=2) as bcp,
            tc.tile_pool(name="yp", bufs=4) as yp,
            tc.tile_pool(name="psum", bufs=1, space="PSUM") as psum,
        ):
            # ---- constants ----
            wqkv_sb = [const.tile([128, 2 * DIM], BF16, tag=f"wqkv{i}", name=f"wqkv{i}")
                       for i in range(2)]
            for i in range(2):
                nc.sync.dma_start(out=wqkv_sb[i], in_=wqkvT_d.ap()[128 * i:128 * (i + 1), :])
            wv_sb = [const.tile([128, VE_W], BF16, tag=f"wv{i}", name=f"wv{i}")
                     for i in range(2)]
            for i in range(2):
                nc.sync.dma_start(out=wv_sb[i], in_=wvT_d.ap()[128 * i:128 * (i + 1), :])
            wp_sb = [const.tile([128, DIM], BF16, tag=f"wp{i}", name=f"wp{i}")
                     for i in range(4)]
            for i in range(4):
                nc.sync.dma_start(out=wp_sb[i], in_=wpT_d.ap()[i])
            mask_sb = [const.tile([128, 512], BF16, tag=f"mask{c}", name=f"mask{c}")
                       for c in range(NCHUNK)]
            for c in range(NCHUNK):
                nc.sync.dma_start(out=mask_sb[c], in_=maskc_d.ap()[c])
            ones_sel = const.tile([128, 33], BF16, tag="ones_sel", name="ones_sel")
            nc.vector.memset(ones_sel, 1.0)
            # [1, VE_W] row: 1.0 at each head's ones column, else 0
            vsel = const.tile([1, VE_W], BF16, tag="vsel", name="vsel")
            nc.vector.memset(vsel, 0.0)
            nc.vector.memset(vsel[:, 32:VE_W:33], 1.0)
            ones_row = const.tile([1, 128], BF16, tag="ones_row", name="ones_row")
            nc.vector.memset(ones_row, 1.0)

            # mm/od/bcast PSUM tags ([128,512] f32 = 1 bank each; sc = 2 banks).
            # One-time zero of od+bcast buffers so never-written partitions
            # (33-63, 97-127) hold finite values (norm-mul reads them; wp rows
            # for them are zero, but NaN*0 would poison proj).
            for i in range(2):
                t = psum.tile([128, 512], F32, tag="od", name="od_init", bufs=2)
                nc.vector.memset(t, 0.0)
            for i in range(2):
                t = psum.tile([128, 512], F32, tag="bcast", name="bc_init", bufs=2)
                nc.vector.memset(t, 0.0)

            mm_rr = [0]

            def mm_tile():
                # rotate qkv/v/proj matmul outputs over the od + bcast banks
                mm_rr[0] += 1
                if mm_rr[0] % 2 == 0:
                    return psum.tile([128, 512], F32, tag="bcast", name="mm_t", bufs=2)
                return psum.tile([128, 512], F32, tag="od", name="mm_t", bufs=2)

            def qkv_phase(b):
                """qkT + Ve matmuls for batch b; returns (qkv, vt) SBUF tiles."""
                xt_sb = [xtp.tile([128, N_TOK], BF16, tag="xt", name="xt_sb")
                         for _ in range(2)]
                for kc in range(2):
                    nc.sync.dma_start(out=xt_sb[kc], in_=xt[b, 128 * kc:128 * (kc + 1), :])
                qkv = [qkvp.tile([128, N_TOK], BF16, tag="qkv", name="qkv_sb")
                       for _ in range(4)]
                for m in range(4):
                    for nh in range(2):
                        ps = mm_tile()
                        for kc in range(2):
                            nc.tensor.matmul(
                                ps,
                                wqkv_sb[kc][:, 128 * m:128 * (m + 1)],
                                xt_sb[kc][:, 512 * nh:512 * (nh + 1)],
                                start=(kc == 0), stop=(kc == 1),
                            )
                        nc.scalar.copy(qkv[m][:, 512 * nh:512 * (nh + 1)], ps)
                vt = [vp.tile([128, VE_W], BF16, tag="v", name="v_sb")
                      for _ in range(NCHUNK)]
                for t in range(NCHUNK):
                    ps = mm_tile()
                    for kc in range(2):
                        nc.tensor.matmul(
                            ps[:, :VE_W], xt_sb[kc][:, 128 * t:128 * (t + 1)],
                            wv_sb[kc],
                            start=(kc == 0), stop=False,
                        )
                    nc.tensor.matmul(ps[:, :VE_W], ones_row, vsel,
                                     start=False, stop=True)
                    nc.scalar.copy(vt[t], ps[:, :VE_W])
                return qkv, vt

            def attention(qkv, vt):
                """scores/exp/mask + PV + normalize; returns aT tiles."""
                pts = [None] * NCHUNK

                def produce(c):
                    qs, wc = _qband(c)
                    pt = ptp.tile([128, HEADS, 512], BF16, tag="pt", name="pt_t")
                    pts[c] = pt
                    for g in range(2):
                        for p in range(2):
                            sc = psum.tile([128, 2, 512], F32, tag="sc",
                                           name="sc_t", bufs=2)
                            for jj in range(2):
                                j = 2 * p + jj
                                nc.tensor.matmul(
                                    sc[:, jj, :wc],
                                    qkv[2 + g][32 * j:32 * (j + 1), 128 * c:128 * (c + 1)],
                                    qkv[0 + g][32 * j:32 * (j + 1), qs:qs + wc],
                                    start=True, stop=True,
                                    tile_position=(32 * j, 0),
                                )
                            hh = 4 * g + 2 * p
                            nc.scalar.activation(pt[:, hh:hh + 2, :wc],
                                                 sc[:, :, :wc], AF.Exp)
                            m = mask_sb[c][:, :wc]
                            mb = bass.AP(tensor=m.tensor, offset=m.offset,
                                         ap=[m.ap[0], [0, 2], m.ap[1]])
                            nc.vector.tensor_mul(pt[:, hh:hh + 2, :wc],
                                                 pt[:, hh:hh + 2, :wc], mb)

                aT = [atp.tile([128, 2, N_TOK], BF16, tag="aT", name="aT_sb")
                      for _ in range(2)]
                produced = [0]
                for h in range(2):
                    cs = _half_chunks(h)
                    for g in range(2):
                        od = [psum.tile([128, 512], F32, tag="od", name="od_t",
                                        bufs=2) for _ in range(2)]
                        for ci, c in enumerate(cs):
                            while produced[0] <= c:
                                produce(produced[0])
                                produced[0] += 1
                            qs, wc = _qband(c)
                            lo = max(512 * h, qs)
                            hi = min(512 * h + 512, qs + wc)
                            po, oo, nw = lo - qs, lo - 512 * h, hi - lo
                            first, last = ci == 0, ci == len(cs) - 1
                            for s in range(2):
                                for jj in range(2):
                                    head = 4 * g + 2 * s + jj
                                    nc.tensor.matmul(
                                        od[s][64 * jj:64 * jj + 33, oo:oo + nw],
                                        vt[c][:, 33 * head:33 * head + 33],
                                        pts[c][:, head, po:po + nw],
                                        start=first, stop=last,
                                        tile_position=(0, 64 * jj),
                                        skip_group_check=True,
                                    )
                        for s in range(2):
                            rc = rcp.tile([128, 512], BF16, tag="rc", name="rc_t")
                            with nc.allow_low_precision(
                                    reason="bf16 recip feeds bf16 PE bcast"):
                                nc.vector.reciprocal(rc, od[s])
                            # stage od in SBUF (frees the PSUM bank early;
                            # TensorTensor may read only one PSUM operand)
                            od_sb = bcp.tile([128, 512], BF16, tag="od_sb",
                                             name="od_sb")
                            nc.scalar.copy(od_sb, od[s])
                            bc = psum.tile([128, 512], F32, tag="bcast",
                                           name="bc_t", bufs=2)
                            nc.tensor.matmul(
                                bc[0:33, :], ones_sel[32:33, :],
                                rc[32:33, :],
                                start=True, stop=True, tile_position=(32, 0),
                                skip_group_check=True,
                            )
                            nc.tensor.matmul(
                                bc[64:97, :], ones_sel[96:97, :],
                                rc[96:97, :],
                                start=True, stop=True, tile_position=(96, 64),
                                skip_group_check=True,
                            )
                            nc.vector.tensor_mul(
                                aT[g][:, s, 512 * h:512 * h + 512], od_sb, bc)
                return aT

            loop_cm = tc.For_i(0, loop_n, 1) if loop_n > 1 else contextlib.nullcontext()
            with loop_cm:
                # software pipeline: qkv(b+1) emitted between attention(b)
                # and proj(b) so PE fills normalize/proj stalls with b+1 work
                st = qkv_phase(0)
                for b in range(B_LOC):
                    aT = attention(*st)
                    if b + 1 < B_LOC:
                        st = qkv_phase(b + 1)

                    # ---- proj: y[tok,256] = aT.T @ wp_e (bias folded in) ----
                    for t in range(NCHUNK):
                        ps = mm_tile()
                        for g in range(2):
                            for s in range(2):
                                nc.tensor.matmul(
                                    ps[:, :DIM], aT[g][:, s, 128 * t:128 * (t + 1)],
                                    wp_sb[2 * g + s],
                                    start=(g == 0 and s == 0), stop=(g == 1 and s == 1),
                                )
                        yt = yp.tile([128, DIM], F32, tag="y", name="y_sb")
                        nc.vector.tensor_copy(yt, ps[:, :DIM])
                        nc.sync.dma_start(out=y[b, 128 * t:128 * (t + 1), :], in_=yt)

    nc.finalize()
    return nc


_PROGRAM = None


def _get_program():
    global _PROGRAM
    if _PROGRAM is None:
        _PROGRAM = build_program()
    return _PROGRAM


def _prep_inputs(x, w_qkv, w_proj, b_proj, mask):
    """Host-side prep: shard, transpose, cast, fold scale/bias, compact mask."""
    scale = HDIM ** -0.5
    wT = np.asarray(w_qkv, np.float32).T.copy()          # [256, 768]
    wT[:, :DIM] *= scale                                 # fold qk scale into q
    wqkvT = wT[:, :2 * DIM].astype(ml_dtypes.bfloat16)   # q,k part

    # Wv^T with a zero column after each head (ones written on-chip)
    wvT = np.zeros((DIM, VE_W), np.float32)
    for hd in range(HEADS):
        wvT[:, 33 * hd:33 * hd + 32] = wT[:, 2 * DIM + 32 * hd:2 * DIM + 32 * hd + 32]
    wvT = wvT.astype(ml_dtypes.bfloat16)

    # wp tiles for the 4 aT lhsT slabs (g,s): rows 0-31 head(4g+2s),
    # row 32 bias (only tile 0; aT row 32 is exactly 1.0), 64-95 head(4g+2s+1).
    wpT = np.asarray(w_proj, np.float32).T               # [256 hd, 256 out]
    wp_e = np.zeros((4, 128, DIM), np.float32)
    for g in range(2):
        for s in range(2):
            i = 2 * g + s
            h_even, h_odd = 4 * g + 2 * s, 4 * g + 2 * s + 1
            wp_e[i, 0:32] = wpT[32 * h_even:32 * h_even + 32]
            wp_e[i, 64:96] = wpT[32 * h_odd:32 * h_odd + 32]
    wp_e[0, 32] = np.asarray(b_proj, np.float32)
    wp_e = wp_e.astype(ml_dtypes.bfloat16)

    m4 = np.asarray(mask, np.float32).reshape(N_TOK, N_TOK)  # [q, k] additive
    maskc = np.zeros((NCHUNK, 128, 512), np.float32)
    for c in range(NCHUNK):
        qs, wc = _qband(c)
        maskc[c, :, :wc] = (m4[qs:qs + wc, 128 * c:128 * (c + 1)] == 0.0).T
    maskc = maskc.astype(ml_dtypes.bfloat16)

    x = np.asarray(x, np.float32)
    in_maps = []
    for core in range(N_CORES):
        xs = x[core * B_LOC:(core + 1) * B_LOC]          # [4, 1024, 256]
        xtl = np.ascontiguousarray(xs.transpose(0, 2, 1)).astype(ml_dtypes.bfloat16)
        in_maps.append({"xt": xtl, "wqkvT": wqkvT, "wvT": wvT, "wpT": wp_e,
                        "maskc": maskc})
    return in_maps


def run(inputs, trace=False):
    nc = _get_program()
    in_maps = _prep_inputs(**inputs)
    res = bass_utils.run_bass_kernel_spmd(
        nc, in_maps, core_ids=list(range(N_CORES)), trace=trace,
    )
    out = np.concatenate([res.results[i]["y"] for i in range(N_CORES)], axis=0)
    return out, res


def kernel(**inputs) -> np.ndarray:
    out, _ = run(inputs, trace=False)
    return out


# revision 20
# speedup vs baseline: 1.2960x; 1.2960x over previous
"""Sparse (sliding-window) attention Trainium2 kernel, v2.

Problem (hardcoded): B=32, N=1024 tokens on a 16x64 grid, C=256, 8 heads,
head_dim=32, 7x11 sliding window. y = softmax(q k^T/sqrt(d) + mask) v @ Wp + b.
Data-parallel over batch: 4 items per core on 8 cores.

v2 changes vs v1 (267us baseline):
  - Softmax denominator folded into the PV matmul: V tiles carry a ones
    column per head (lhsT [128,33], two heads per PSUM od tile at
    tile_position (0,0)/(0,64)), so the separate ones-lhsT denominator
    matmuls (half the PV stream) disappear.
  - PV accumulates over q-halves (512 wide) instead of quarters: one
    matmul per (chunk, head) half-overlap, start=True on the first chunk
    (whole-bank clear replaces DVE memsets of od).
  - Normalization: one DVE reciprocal per od tile, K=1 selector matmuls
    broadcast the den-reciprocal rows (32/96) across each head's 32
    output partitions (M=33 so the den row itself becomes exactly 1.0 in
    aT), one DVE multiply od*bcast -> aT. The 1.0 rows let the proj bias
    fold into wp (row 32 of the (g=0,s=0) tile), removing bias matmuls.
  - aT keeps junk partitions (33-63, 97-127); host-side wp rows for those
    are zero. One-time PSUM memsets at init keep the junk finite.
  - PSUM->SBUF copies moved off DVE: qkv/v on ScalarE, y on GpSimd.
"""

import contextlib

import numpy as np
import ml_dtypes

import concourse.bass as bass
import concourse.bacc as bacc
import concourse.mybir as mybir
import concourse.tile as tile
from concourse import bass_utils

F32 = mybir.dt.float32
BF16 = mybir.dt.bfloat16
AF = mybir.ActivationFunctionType

H_MAP, W_MAP = 16, 64
N_TOK = H_MAP * W_MAP            # 1024
DIM = 256
HEADS = 8
HDIM = 32
B_FULL = 32
N_CORES = 8
B_LOC = B_FULL // N_CORES        # 4
NCHUNK = N_TOK // 128            # 8 k-chunks (2 grid rows each)
HALF = 512
VE_W = 33 * HEADS                # 264: per-head 32 V dims + ones col


def _qband(c):
    """Valid q range (start token, width) for k-chunk c (rows 2c, 2c+1)."""
    qlo = max(0, 2 * c - 3)
    qhi = min(H_MAP - 1, 2 * c + 4)
    return qlo * W_MAP, (qhi - qlo + 1) * W_MAP


def _half_chunks(h):
    out = []
    for c in range(NCHUNK):
        qs, wc = _qband(c)
        if qs < 512 * (h + 1) and qs + wc > 512 * h:
            out.append(c)
    return out


def build_program(loop_n=1):
    nc = bacc.Bacc("TRN2", target_bir_lowering=False, debug=False)

    xt_d = nc.dram_tensor("xt", [B_LOC, DIM, N_TOK], BF16, kind="ExternalInput")
    wqkvT_d = nc.dram_tensor("wqkvT", [DIM, 2 * DIM], BF16, kind="ExternalInput")
    wvT_d = nc.dram_tensor("wvT", [DIM, VE_W], BF16, kind="ExternalInput")
    wpT_d = nc.dram_tensor("wpT", [4, 128, DIM], BF16, kind="ExternalInput")
    maskc_d = nc.dram_tensor("maskc", [NCHUNK, 128, 512], BF16, kind="ExternalInput")
    y_d = nc.dram_tensor("y", [B_LOC, N_TOK, DIM], F32, kind="ExternalOutput")

    xt = xt_d.ap()
    y = y_d.ap()

    with tile.TileContext(nc) as tc:
        with (
            tc.tile_pool(name="const", bufs=1) as const,
            tc.tile_pool(name="xtp", bufs=4) as xtp,
            tc.tile_pool(name="qkvp", bufs=8) as qkvp,
            tc.tile_pool(name="vp", bufs=16) as vp,
            tc.tile_pool(name="ptp", bufs=8) as ptp,
            tc.tile_pool(name="atp", bufs=4) as atp,
            tc.tile_pool(name="rcp", bufs=4) as rcp,
            tc.tile_pool(name="bcp", bufs=4) as bcp,
            tc.tile_pool(name="yp", bufs=4) as yp,
            tc.tile_pool(name="psum", bufs=1, space="PSUM") as psum,
        ):
            # ---- constants ----
            wqkv_sb = [const.tile([128, 2 * DIM], BF16, tag=f"wqkv{i}", name=f"wqkv{i}")
                       for i in range(2)]
            for i in range(2):
                nc.sync.dma_start(out=wqkv_sb[i], in_=wqkvT_d.ap()[128 * i:128 * (i + 1), :])
            wv_sb = [const.tile([128, VE_W], BF16, tag=f"wv{i}", name=f"wv{i}")
                     for i in range(2)]
            for i in range(2):
                nc.sync.dma_start(out=wv_sb[i], in_=wvT_d.ap()[128 * i:128 * (i + 1), :])
            wp_sb = [const.tile([128, DIM], BF16, tag=f"wp{i}", name=f"wp{i}")
                     for i in range(4)]
            for i in range(4):
                nc.sync.dma_start(out=wp_sb[i], in_=wpT_d.ap()[i])
            mask_sb = [const.tile([128, 512], BF16, tag=f"mask{c}", name=f"mask{c}")
                       for c in range(NCHUNK)]
            for c in range(NCHUNK):
                nc.sync.dma_start(out=mask_sb[c], in_=maskc_d.ap()[c])
            ones_sel = const.tile([128, 33], BF16, tag="ones_sel", name="ones_sel")
            nc.vector.memset(ones_sel, 1.0)
            # [1, VE_W] row: 1.0 at each head's ones column, else 0
            vsel = const.tile([1, VE_W], BF16, tag="vsel", name="vsel")
            nc.vector.memset(vsel, 0.0)
            nc.vector.memset(vsel[:, 32:VE_W:33], 1.0)
            ones_row = const.tile([1, 128], BF16, tag="ones_row", name="ones_row")
            nc.vector.memset(ones_row, 1.0)

            # mm/od/bcast PSUM tags ([128,512] f32 = 1 bank each; sc = 2 banks).
            # One-time zero of od+bcast buffers so never-written partitions
            # (33-63, 97-127) hold finite values (norm-mul reads them; wp rows
            # for them are zero, but NaN*0 would poison proj).
            for i in range(3):
                t = psum.tile([128, 512], F32, tag="od", name="od_init", bufs=3)
                nc.vector.memset(t, 0.0)
            t = psum.tile([128, 512], F32, tag="bcast", name="bc_init", bufs=1)
            nc.vector.memset(t, 0.0)

            mm_rr = [0]

            def mm_tile():
                # rotate qkv/v/proj matmul outputs over the od + bcast banks
                mm_rr[0] += 1
                if mm_rr[0] % 2 == 0:
                    return psum.tile([128, 512], F32, tag="bcast", name="mm_t", bufs=1)
                return psum.tile([128, 512], F32, tag="od", name="mm_t", bufs=3)

            def qkv_phase(b):
                """qkT + Ve matmuls for batch b; returns (qkv, vt) SBUF tiles."""
                xt_sb = [xtp.tile([128, N_TOK], BF16, tag="xt", name="xt_sb")
                         for _ in range(2)]
                for kc in range(2):
                    nc.sync.dma_start(out=xt_sb[kc], in_=xt[b, 128 * kc:128 * (kc + 1), :])
                qkv = [qkvp.tile([128, N_TOK], BF16, tag="qkv", name="qkv_sb")
                       for _ in range(4)]
                for m in range(4):
                    for nh in range(2):
                        ps = mm_tile()
                        for kc in range(2):
                            nc.tensor.matmul(
                                ps,
                                wqkv_sb[kc][:, 128 * m:128 * (m + 1)],
                                xt_sb[kc][:, 512 * nh:512 * (nh + 1)],
                                start=(kc == 0), stop=(kc == 1),
                            )
                        nc.scalar.copy(qkv[m][:, 512 * nh:512 * (nh + 1)], ps)
                vt = [vp.tile([128, VE_W], BF16, tag="v", name="v_sb")
                      for _ in range(NCHUNK)]
                for t in range(NCHUNK):
                    ps = mm_tile()
                    for kc in range(2):
                        nc.tensor.matmul(
                            ps[:, :VE_W], xt_sb[kc][:, 128 * t:128 * (t + 1)],
                            wv_sb[kc],
                            start=(kc == 0), stop=False,
                        )
                    nc.tensor.matmul(ps[:, :VE_W], ones_row, vsel,
                                     start=False, stop=True)
                    nc.scalar.copy(vt[t], ps[:, :VE_W])
                return qkv, vt

            def attention(qkv, vt, bg=()):
                """scores/exp/mask + PV + normalize; returns aT tiles.
                bg: deferred proj-tile emitters of the previous batch,
                interleaved after each (half, g) block to fill stalls."""
                bg = list(bg)
                pts = [None] * NCHUNK

                def produce(c):
                    qs, wc = _qband(c)
                    pt = ptp.tile([128, HEADS, 512], BF16, tag="pt", name="pt_t")
                    pts[c] = pt
                    for g in range(2):
                        for p in range(2):
                            sc = psum.tile([128, 2, 512], F32, tag="sc",
                                           name="sc_t", bufs=2)
                            for jj in range(2):
                                j = 2 * p + jj
                                nc.tensor.matmul(
                                    sc[:, jj, :wc],
                                    qkv[2 + g][32 * j:32 * (j + 1), 128 * c:128 * (c + 1)],
                                    qkv[0 + g][32 * j:32 * (j + 1), qs:qs + wc],
                                    start=True, stop=True,
                                    tile_position=(32 * j, 0),
                                )
                            hh = 4 * g + 2 * p
                            nc.scalar.activation(pt[:, hh:hh + 2, :wc],
                                                 sc[:, :, :wc], AF.Exp)
                            m = mask_sb[c][:, :wc]
                            mb = bass.AP(tensor=m.tensor, offset=m.offset,
                                         ap=[m.ap[0], [0, 2], m.ap[1]])
                            nc.vector.tensor_mul(pt[:, hh:hh + 2, :wc],
                                                 pt[:, hh:hh + 2, :wc], mb)

                aT = [atp.tile([128, 2, N_TOK], BF16, tag="aT", name="aT_sb")
                      for _ in range(2)]
                produced = [0]
                for h in range(2):
                    cs = _half_chunks(h)
                    for g in range(2):
                        od = [psum.tile([128, 512], F32, tag="od", name="od_t",
                                        bufs=3) for _ in range(2)]
                        for ci, c in enumerate(cs):
                            while produced[0] <= c:
                                produce(produced[0])
                                produced[0] += 1
                            qs, wc = _qband(c)
                            lo = max(512 * h, qs)
                            hi = min(512 * h + 512, qs + wc)
                            po, oo, nw = lo - qs, lo - 512 * h, hi - lo
                            first, last = ci == 0, ci == len(cs) - 1
                            for s in range(2):
                                for jj in range(2):
                                    head = 4 * g + 2 * s + jj
                                    nc.tensor.matmul(
                                        od[s][64 * jj:64 * jj + 33, oo:oo + nw],
                                        vt[c][:, 33 * head:33 * head + 33],
                                        pts[c][:, head, po:po + nw],
                                        start=first, stop=last,
                                        tile_position=(0, 64 * jj),
                                        skip_group_check=True,
                                    )
                        for s in range(2):
                            # stage od in SBUF first: frees the PSUM bank early,
                            # recip then runs all-SBUF bf16 (2x), and the
                            # norm-mul has a single PSUM operand (bc)
                            od_sb = bcp.tile([128, 512], BF16, tag="od_sb",
                                             name="od_sb")
                            nc.vector.tensor_copy(od_sb, od[s])
                            rc = rcp.tile([128, 512], BF16, tag="rc", name="rc_t")
                            with nc.allow_low_precision(
                                    reason="bf16 recip feeds bf16 PE bcast"):
                                nc.vector.reciprocal(rc, od_sb)
                            bc = psum.tile([128, 512], F32, tag="bcast",
                                           name="bc_t", bufs=1)
                            nc.tensor.matmul(
                                bc[0:33, :], ones_sel[32:33, :],
                                rc[32:33, :],
                                start=True, stop=True, tile_position=(32, 0),
                                skip_group_check=True,
                            )
                            nc.tensor.matmul(
                                bc[64:97, :], ones_sel[96:97, :],
                                rc[96:97, :],
                                start=True, stop=True, tile_position=(96, 64),
                                skip_group_check=True,
                            )
                            nc.vector.tensor_mul(
                                aT[g][:, s, 512 * h:512 * h + 512], od_sb, bc)
                        for _ in range(2):
                            if bg:
                                bg.pop(0)()
                while bg:
                    bg.pop(0)()
                return aT

            loop_cm = tc.For_i(0, loop_n, 1) if loop_n > 1 else contextlib.nullcontext()
            with loop_cm:
                # software pipeline: qkv(b+1) emitted between attention(b)
                # and proj(b) so PE fills normalize/proj stalls with b+1 work
                st = qkv_phase(0)
                for b in range(B_LOC):
                    aT = attention(*st)
                    if b + 1 < B_LOC:
                        st = qkv_phase(b + 1)
                    for t in range(NCHUNK):
                        ps = mm_tile()
                        for g in range(2):
                            for s in range(2):
                                nc.tensor.matmul(
                                    ps[:, :DIM], aT[g][:, s, 128 * t:128 * (t + 1)],
                                    wp_sb[2 * g + s],
                                    start=(g == 0 and s == 0), stop=(g == 1 and s == 1),
                                )
                        yt = yp.tile([128, DIM], F32, tag="y", name="y_sb")
                        nc.vector.tensor_copy(yt, ps[:, :DIM])
                        nc.sync.dma_start(out=y[b, 128 * t:128 * (t + 1), :], in_=yt)

    nc.finalize()
    return nc


_PROGRAM = None


def _get_program():
    global _PROGRAM
    if _PROGRAM is None:
        _PROGRAM = build_program()
    return _PROGRAM


def _prep_inputs(x, w_qkv, w_proj, b_proj, mask):
    """Host-side prep: shard, transpose, cast, fold scale/bias, compact mask."""
    scale = HDIM ** -0.5
    wT = np.asarray(w_qkv, np.float32).T.copy()          # [256, 768]
    wT[:, :DIM] *= scale                                 # fold qk scale into q
    wqkvT = wT[:, :2 * DIM].astype(ml_dtypes.bfloat16)   # q,k part

    # Wv^T with a zero column after each head (ones written on-chip)
    wvT = np.zeros((DIM, VE_W), np.float32)
    for hd in range(HEADS):
        wvT[:, 33 * hd:33 * hd + 32] = wT[:, 2 * DIM + 32 * hd:2 * DIM + 32 * hd + 32]
    wvT = wvT.astype(ml_dtypes.bfloat16)

    # wp tiles for the 4 aT lhsT slabs (g,s): rows 0-31 head(4g+2s),
    # row 32 bias (only tile 0; aT row 32 is exactly 1.0), 64-95 head(4g+2s+1).
    wpT = np.asarray(w_proj, np.float32).T               # [256 hd, 256 out]
    wp_e = np.zeros((4, 128, DIM), np.float32)
    for g in range(2):
        for s in range(2):
            i = 2 * g + s
            h_even, h_odd = 4 * g + 2 * s, 4 * g + 2 * s + 1
            wp_e[i, 0:32] = wpT[32 * h_even:32 * h_even + 32]
            wp_e[i, 64:96] = wpT[32 * h_odd:32 * h_odd + 32]
    wp_e[0, 32] = np.asarray(b_proj, np.float32)
    wp_e = wp_e.astype(ml_dtypes.bfloat16)

    m4 = np.asarray(mask, np.float32).reshape(N_TOK, N_TOK)  # [q, k] additive
    maskc = np.zeros((NCHUNK, 128, 512), np.float32)
    for c in range(NCHUNK):
        qs, wc = _qband(c)
        maskc[c, :, :wc] = (m4[qs:qs + wc, 128 * c:128 * (c + 1)] == 0.0).T
    maskc = maskc.astype(ml_dtypes.bfloat16)

    x = np.asarray(x, np.float32)
    in_maps = []
    for core in range(N_CORES):
        xs = x[core * B_LOC:(core + 1) * B_LOC]          # [4, 1024, 256]
        xtl = np.ascontiguousarray(xs.transpose(0, 2, 1)).astype(ml_dtypes.bfloat16)
        in_maps.append({"xt": xtl, "wqkvT": wqkvT, "wvT": wvT, "wpT": wp_e,
                        "maskc": maskc})
    return in_maps


def run(inputs, trace=False):
    nc = _get_program()
    in_maps = _prep_inputs(**inputs)
    res = bass_utils.run_bass_kernel_spmd(
        nc, in_maps, core_ids=list(range(N_CORES)), trace=trace,
    )
    out = np.concatenate([res.results[i]["y"] for i in range(N_CORES)], axis=0)
    return out, res


def kernel(**inputs) -> np.ndarray:
    out, _ = run(inputs, trace=False)
    return out
